# revision 61
# baseline (speedup 1.0000x reference)
"""Object-condensation loss on 8 Trainium2 NeuronCores (Bass/Tile SPMD).

Hits are sharded 6250/core. The segment-max/argmax center search runs
replicated on every core over the unique-edge grid (f and hit-id rows
in f32 for an exact argmax; x rows in bf16 with edge-slots on
partitions). The winning-slot one-hot mask is PE-transposed so the
center-coordinate selection contracts on the tensor engine via
ones-vector matmuls. The dense [hits x 512] hinge term runs on PE
(f32r matmuls, one PSUM bank per 128-hit tile) with fused one-pass
consumers: ACT relu+scale+accumulate on single banks, DVE
(ps min 0)*(-q) scalar_tensor_tensor with accumulate over 3-bank
groups (GPSIMD cannot touch PSUM, so it handles the SBUF-side work:
mask TTs, member-correction products, background terms). Per-core
accumulators and partial scalars are DMA'd out raw and combined on
the host (the unshard step).

Host-side work is strictly index-driven data LAYOUT (slicing, padding,
reshapes, staging rows of [x|f] into edge grids by integer edge
indices, dtype casts) -- no floating-point arithmetic on the host.
"""
import os
import sys

sys.path.insert(0, "/opt/trn_rl_repo")

import numpy as np
import ml_dtypes

import concourse.bass as bass
import concourse.bacc as bacc
import concourse.tile as tile
from concourse import mybir
from concourse.bass_utils import run_bass_kernel_spmd
import jax as _jax
try:
    os.makedirs(os.path.expanduser("~/.cache/bass_jax_cache"), exist_ok=True)
    _jax.config.update("jax_compilation_cache_dir",
                       os.path.expanduser("~/.cache/bass_jax_cache"))
    _jax.config.update("jax_persistent_cache_min_entry_size_bytes", 0)
    _jax.config.update("jax_persistent_cache_min_compile_time_secs", 0)
except Exception:
    pass
from concourse.masks import make_identity

P = 128
NC = 8
N_HIT = 50000
N_TRUE = 512
D = 8
S_B = 1.0
Q_MIN = 0.5
HPC = N_HIT // NC          # 6250 hits per core
G49 = 49                   # free tiles per partition: 128*49 = 6272
HPAD = P * G49             # padded hits per core
NT4 = N_TRUE // P          # 4 segment tiles
F32 = mybir.dt.float32
F32R = mybir.dt.float32r
BF16 = mybir.dt.bfloat16
AX = mybir.AxisListType
OP = mybir.AluOpType
ACTF = mybir.ActivationFunctionType

N_WARM = 14                # PE p-state warmup matmuls

def build_kernel(LF, LC):
    """LF: slots/segment in the replicated center grid.
    LC: slots/segment in the per-core member-correction grid."""
    nc = bacc.Bacc("TRN2", target_bir_lowering=False, debug=False,
                   num_devices=NC)
    vec = nc.vector
    act = nc.scalar
    gps = nc.gpsimd

    # ---------------- I/O ----------------
    gf_in = nc.dram_tensor("gf", [P, NT4, LF], F32, kind="ExternalInput")
    gh_in = nc.dram_tensor("gh", [P, NT4, LF], F32, kind="ExternalInput")
    gxt_in = nc.dram_tensor("gxt", [LF, D, NT4, P], BF16, kind="ExternalInput")
    xt9_in = nc.dram_tensor("xt9", [D + 1, HPAD], F32R, kind="ExternalInput")
    xpm_in = nc.dram_tensor("xpm", [P, G49 * D], F32, kind="ExternalInput")
    fpm_in = nc.dram_tensor("fpm", [P, G49], F32, kind="ExternalInput")
    ypm_in = nc.dram_tensor("ypm", [P, G49], F32, kind="ExternalInput")
    nmask_in = nc.dram_tensor("nmask", [P, G49], F32, kind="ExternalInput")
    gg_in = nc.dram_tensor("gg", [P, NT4, LC, D + 1], F32, kind="ExternalInput")
    part_out = nc.dram_tensor("part", [P, NC], F32, kind="ExternalOutput")
    vacc_out = nc.dram_tensor("vacc", [P, 3, G49], F32, kind="ExternalOutput")

    with tile.TileContext(nc) as tc:
        with (
            tc.tile_pool(name="sbuf", bufs=1) as sb,
            tc.tile_pool(name="sjunk", bufs=2) as sj,
            tc.tile_pool(name="psum", bufs=2, space="PSUM") as pp,
            tc.tile_pool(name="psa", bufs=2, space="PSUM") as pa,
        ):
            # ------------- loads (gpsimd queue: cheap issue) -------------
            gf = sb.tile([P, NT4, LF], F32)
            nc.sync.dma_start(out=gf[:], in_=gf_in[:])
            gh = sb.tile([P, NT4, LF], F32)
            nc.sync.dma_start(out=gh[:], in_=gh_in[:])
            ident = sb.tile([P, P], F32)
            make_identity(nc, ident[:])
            cm1 = sb.tile([P, 1], F32)
            gps.memset(cm1[:], -1.0)
            actwarm = sb.tile([P, 1], F32)
            act.activation(out=actwarm[:], in_=cm1[:], func=ACTF.Ln,
                           scale=-1.0)
            gxt = sb.tile([LF, D, NT4, P], BF16)
            nc.sync.dma_start(out=gxt[:], in_=gxt_in[:])
            onesb = sb.tile([LF, 1], BF16)
            gps.memset(onesb[:], 1.0)
            xt = sb.tile([D + 2, HPAD], F32R)
            nc.sync.dma_start(out=xt[0:D + 1, :], in_=xt9_in[:])
            xpm = sb.tile([P, G49, D], F32)
            nc.sync.dma_start(out=xpm[:],
                              in_=xpm_in[:].rearrange("p (g d) -> p g d", d=D))
            fpm = sb.tile([P, G49], F32)
            nc.sync.dma_start(out=fpm[:], in_=fpm_in[:])
            ypm = sb.tile([P, G49], F32)
            nc.sync.dma_start(out=ypm[:], in_=ypm_in[:])
            nmask = sb.tile([P, G49], F32)
            nc.sync.dma_start(out=nmask[:], in_=nmask_in[:])
            gg = sb.tile([P, NT4, LC, D + 1], F32)
            nc.sync.dma_start(out=gg[:], in_=gg_in[:])

            # ------------- PE p-state warmup (small f32 matmuls) -------------
            for w in range(N_WARM):
                wp = pa.tile([P, 512], F32, space="PSUM", tag="da")
                nc.tensor.matmul(out=wp[:, 0:32], lhsT=ident[:],
                                 rhs=ident[:, 0:32], start=True, stop=True)

            # ------------- center search: segment max + argmax -------------
            cfe = gf[:]                                 # [P, 4, LF] f
            chp = gh[:]                                 # [P, 4, LF] hit+1
            fmax = sb.tile([P, NT4], F32)
            vec.tensor_reduce(out=fmax[:], in_=cfe, axis=AX.X, op=OP.max)
            xsq = sj.tile([P, G49, D], F32, tag="xsq")
            act.activation(out=xsq[:], in_=xpm[:], func=ACTF.Square)
            eq1 = sj.tile([P, NT4, LF], F32, tag="eq1")
            vec.tensor_tensor(out=eq1[:], in0=cfe,
                              in1=fmax[:].to_broadcast([P, NT4, LF]),
                              op=OP.is_equal)
            cnd = sj.tile([P, NT4, LF], F32, tag="cnd")
            vec.tensor_tensor(out=cnd[:], in0=eq1[:], in1=chp, op=OP.mult)
            candp = sb.tile([P, NT4], F32)
            vec.tensor_reduce(out=candp[:], in_=cnd[:], axis=AX.X, op=OP.max)
            m3 = sb.tile([P, NT4, LF], F32)
            vec.tensor_tensor(out=m3[:], in0=chp,
                              in1=candp[:].to_broadcast([P, NT4, LF]),
                              op=OP.is_equal)

            # qc chain (ACT-heavy, off the DVE critical path)
            qcb = sj.tile([P, NT4], F32, tag="qcb")
            qcr = sj.tile([P, NT4], F32, tag="qcr")
            qcl = sj.tile([P, NT4], F32, tag="qcl")
            qcs = sj.tile([P, NT4], F32, tag="qcs")
            vec.tensor_scalar(qcb[:], fmax[:], -1.0, 1.0, OP.mult, OP.add)
            vec.reciprocal(qcr[:], qcb[:])
            act.activation(out=qcl[:], in_=qcr[:], func=ACTF.Ln,
                           scale=2.0, bias=cm1[:])
            act.activation(out=qcs[:], in_=qcl[:], func=ACTF.Square, scale=0.5)
            qc = sb.tile([P, NT4], F32)
            vec.tensor_scalar(qc[:], qcs[:], Q_MIN, None, OP.add)
            n2qc = sb.tile([P, NT4], F32)
            vec.tensor_scalar(n2qc[:], qc[:], -2.0, None, OP.mult)
            fcsum = sb.tile([P, 1], F32)
            vec.tensor_reduce(out=fcsum[:], in_=fmax[:], axis=AX.X, op=OP.add)

            # d-selection: transpose m3 to [LF, NT4*P]; mask the
            # slot-major x grid (bf16 2x TTs); contract slots on PE via
            # ones-vector matmuls -> xc[seg, t*8+d] lands on partitions.
            m3w3 = pp.tile([P, 3, 512], F32, space="PSUM", tag="dense")
            m3ps = m3w3[0:LF, 0, :]
            for t in range(NT4):
                nc.tensor.transpose(out=m3ps[:, t * P:(t + 1) * P],
                                    in_=m3[:, t, :], identity=ident[:])
            m3t = sb.tile([LF, NT4, P], BF16)
            vec.tensor_copy(out=m3t[:], in_=m3ps.rearrange(
                "l (t p) -> l t p", p=P))
            m3v = m3t[:]
            xcw = pa.tile([P, 512], F32, space="PSUM", tag="da")
            x2pm = sb.tile([P, G49], F32)
            x2sb = sb.tile([G49, P], F32R)
            for d in range(D):
                md = sj.tile([LF, NT4, P], BF16, tag=f"md{d % 4}")
                eng = gps if d < 2 else vec
                eng.tensor_tensor(out=md[:], in0=gxt[:, d, :, :], in1=m3v,
                                  op=OP.mult)
                for t in range(NT4):
                    nc.tensor.matmul(out=xcw[:, t * D + d:t * D + d + 1],
                                     lhsT=md[:, t, :], rhs=onesb[:],
                                     start=True, stop=True)
                if d == 2:
                    vec.tensor_reduce(out=x2pm[:], in_=xsq[:], axis=AX.X,
                                      op=OP.add)
                    x2ps = pa.tile([P, 512], F32, space="PSUM", tag="da")
                    nc.tensor.transpose(out=x2ps[0:G49, 0:P], in_=x2pm[:],
                                        identity=ident[:])
                if d == 3:
                    vec.tensor_copy(out=x2sb[:], in_=x2ps[0:G49, 0:P])
                    nc.sync.dma_start(out=xt[D + 1:D + 2, :], in_=x2sb[:])
            xcf = sb.tile([P, NT4 * D], F32)
            vec.tensor_copy(out=xcf[:], in_=xcw[:, 0:NT4 * D])
            xc = xcf[:].rearrange("p (t d) -> p t d", d=D)

            # ------------- background terms (pool) -------------
            bkg = sj.tile([P, G49], F32, tag="bkg")
            vec.tensor_scalar(bkg[:], ypm[:], -1.0, None, OP.is_equal)
            bf = sj.tile([P, G49], F32, tag="bf")
            b2num = sb.tile([P, 1], F32)
            gps.tensor_tensor(out=bf[:], in0=bkg[:], in1=fpm[:], op=OP.mult)
            vec.tensor_reduce(out=b2num[:], in_=bf[:], axis=AX.X, op=OP.add)
            nbkg = sb.tile([P, 1], F32)
            vec.tensor_reduce(out=nbkg[:], in_=bkg[:], axis=AX.X, op=OP.add)

            # ------------- ctil = [-2qc*xc | qc*(c2-1) | qc] -> cT ------
            csq = sj.tile([P, NT4 * D], F32, tag="csq")
            vec.tensor_tensor(out=csq[:], in0=xcf[:], in1=xcf[:], op=OP.mult)
            c2 = sb.tile([P, NT4], F32)
            vec.tensor_reduce(out=c2[:], in_=csq[:].rearrange(
                "p (t d) -> p t d", d=D), axis=AX.X, op=OP.add)
            ctil = sb.tile([P, NT4, D + 2], F32)
            vec.tensor_tensor(
                out=ctil[:, :, 0:D], in0=xc,
                in1=n2qc[:].to_broadcast([P, NT4, D]), op=OP.mult)
            vec.scalar_tensor_tensor(
                out=ctil[:, :, D:D + 1].rearrange("p t o -> p (t o)"),
                in0=c2[:], scalar=-1.0, in1=qc[:], op0=OP.add, op1=OP.mult)
            vec.tensor_copy(out=ctil[:, :, D + 1:D + 2].rearrange("p t o -> p (t o)"),
                            in_=qc[:])
            ctps = pa.tile([P, 512], F32, space="PSUM", tag="da")
            for t in range(NT4):
                nc.tensor.transpose(out=ctps[0:D + 2, t * P:(t + 1) * P],
                                    in_=ctil[:, t, :], identity=ident[:])
            cT = sb.tile([D + 2, N_TRUE], F32R)
            act.activation(out=cT[:], in_=ctps[0:D + 2, :], func=ACTF.Copy)

            # ------------- per-hit q (negated, masked) -------------
            qb = sj.tile([P, G49], F32, tag="qb")
            qr = sj.tile([P, G49], F32, tag="qr")
            ql = sj.tile([P, G49], F32, tag="ql")
            qs = sj.tile([P, G49], F32, tag="qs")
            vec.tensor_scalar(qb[:], fpm[:], -1.0, 1.0, OP.mult, OP.add)
            vec.reciprocal(qr[:], qb[:])
            act.activation(out=ql[:], in_=qr[:], func=ACTF.Ln,
                           scale=2.0, bias=cm1[:])
            act.activation(out=qs[:], in_=ql[:], func=ACTF.Square, scale=0.5)
            negq = sb.tile([P, G49], F32)
            vec.scalar_tensor_tensor(out=negq[:], in0=qs[:], scalar=Q_MIN,
                                     in1=nmask[:], op0=OP.add, op1=OP.mult)


            # ------------- dense phase: one fused consumer per tile ------
            acc_a = sb.tile([P, G49], F32)
            acc_d = sb.tile([P, 16], F32)
            gps.memset(acc_a[:], 0.0)
            gps.memset(acc_d[:], 0.0)

            # work items: 9 DVE 3-bank groups (27 tiles) + 22 ACT singles
            items = []
            gD, gA = 0, 27
            acounts = [2, 2, 2, 2, 2, 2, 2, 2, 2]
            for _ in range(4):
                items.append(("A", gA, 0))
                gA += 1
            for i in range(9):
                items.append(("D", gD, i))
                gD += 3
                for _ in range(acounts[i]):
                    items.append(("A", gA, 0))
                    gA += 1

            def dense_item(kind, g0, slot):
                if kind == "D":
                    ps3 = pp.tile([P, 3, 512], F32, space="PSUM", tag="dense")
                    for j in range(3):
                        g = g0 + j
                        nc.tensor.matmul(out=ps3[:, j, :],
                                         lhsT=xt[:, g * P:(g + 1) * P],
                                         rhs=cT[:], start=True, stop=True)
                    nqv = negq[:, g0:g0 + 3].rearrange(
                        "p j -> p j ()").to_broadcast([P, 3, N_TRUE])
                    jd = sj.tile([P, 3, N_TRUE], F32, tag="jd")
                    vec.scalar_tensor_tensor(
                        out=jd[:], in0=ps3[:], scalar=0.0, in1=nqv,
                        op0=OP.min, op1=OP.mult,
                        accum_out=acc_d[:, slot:slot + 1])
                else:
                    ps = pa.tile([P, 512], F32, space="PSUM", tag="da")
                    nc.tensor.matmul(out=ps[:], lhsT=xt[:, g0 * P:(g0 + 1) * P],
                                     rhs=cT[:], start=True, stop=True)
                    act.activation(out=ps[:], in_=ps[:], func=ACTF.Relu,
                                   scale=negq[:, g0:g0 + 1],
                                   accum_out=acc_a[:, g0:g0 + 1])

            for kind, g0, slot in items[:8]:
                dense_item(kind, g0, slot)

            # ------------- member correction (mid-dense; d-inner grid) ------
            dif = sj.tile([P, NT4, LC, D], F32, tag="dif")
            vec.tensor_tensor(
                out=dif[:], in0=gg[:, :, :, 0:D],
                in1=xcf[:].rearrange("p (t d) -> p t () d", d=D).to_broadcast(
                    [P, NT4, LC, D]),
                op=OP.subtract)
            dsq = sj.tile([P, NT4, LC, D], F32, tag="dsq")
            vec.tensor_tensor(out=dsq[:], in0=dif[:], in1=dif[:], op=OP.mult)
            dste = sj.tile([P, NT4, LC], F32, tag="dste")
            vec.tensor_reduce(out=dste[:], in_=dsq[:], axis=AX.X, op=OP.add)
            # hinge correction weight: dist + min(dist-1, 0)
            we0 = sj.tile([P, NT4, LC], F32, tag="we0")
            vec.tensor_scalar(we0[:], dste[:], 1.0, 0.0, OP.subtract, OP.min)
            we = sj.tile([P, NT4, LC], F32, tag="we")
            gps.tensor_tensor(out=we[:], in0=we0[:], in1=dste[:], op=OP.add)
            wq = sj.tile([P, NT4, LC], F32, tag="wq")
            gps.tensor_tensor(out=wq[:], in0=we[:],
                              in1=qc[:].to_broadcast([P, NT4, LC]), op=OP.mult)
            # qe = (atanh(fe)^2 + Q_MIN) * (fe >= 0)
            fe = gg[:, :, :, D]                          # [P, 4, LC]
            eb = sj.tile([P, NT4, LC], F32, tag="eb")
            ec = sj.tile([P, NT4, LC], F32, tag="ec")
            er = sj.tile([P, NT4, LC], F32, tag="er")
            el = sj.tile([P, NT4, LC], F32, tag="el")
            es = sj.tile([P, NT4, LC], F32, tag="es")
            ev = sj.tile([P, NT4, LC], F32, tag="ev")
            # clamp pad slots (fe = -1) to f = 0 so Ln stays finite
            vec.tensor_scalar(ec[:], fe, 0.0, -1.0, OP.max, OP.mult)
            vec.tensor_scalar(eb[:], ec[:], 1.0, None, OP.add)
            vec.reciprocal(er[:], eb[:])
            act.activation(out=el[:], in_=er[:], func=ACTF.Ln,
                           scale=2.0, bias=cm1[:])
            act.activation(out=es[:], in_=el[:], func=ACTF.Square, scale=0.5)
            vec.tensor_scalar(ev[:], fe, 0.0, None, OP.is_ge)
            qe = sj.tile([P, NT4, LC], F32, tag="qe")
            vec.scalar_tensor_tensor(out=qe[:], in0=es[:], scalar=Q_MIN,
                                     in1=ev[:], op0=OP.add, op1=OP.mult)
            wfin = sj.tile([P, NT4, LC], F32, tag="wfin")
            corr = sb.tile([P, 1], F32)
            gps.tensor_tensor(out=wfin[:], in0=wq[:], in1=qe[:], op=OP.mult,
                              )
            vec.tensor_reduce(out=corr[:], in_=wfin[:], axis=AX.XY, op=OP.add)

            for kind, g0, slot in items[8:]:
                dense_item(kind, g0, slot)

            # ------------- partial outputs [P, 8] -------------
            stk = sb.tile([P, NC], F32)
            gps.memset(stk[:], 0.0)
            gps.tensor_copy(out=stk[:, 1:2], in_=b2num[:])
            gps.tensor_copy(out=stk[:, 2:3], in_=nbkg[:])
            gps.tensor_copy(out=stk[:, 3:4], in_=fcsum[:])
            gps.tensor_copy(out=stk[:, 4:5], in_=corr[:])
            nc.sync.dma_start(out=part_out[:], in_=stk[:])
            act.dma_start(out=vacc_out[:, 0, 0:G49], in_=acc_a[:])
            nc.sync.dma_start(out=vacc_out[:, 1, 0:16], in_=acc_d[:])

    nc.compile()
    return nc


_CACHE = {}


def _get_kernel(LF, LC):
    key = (LF, LC)
    if key not in _CACHE:
        _CACHE[key] = build_kernel(LF, LC)
    return _CACHE[key]


def _prep(x, f, y, e_h, e_p):
    x = np.asarray(x, np.float32)
    f = np.asarray(f, np.float32)
    y = np.asarray(y).astype(np.int64)
    e_h = np.asarray(e_h).astype(np.int64)
    e_p = np.asarray(e_p).astype(np.int64)

    keys = e_h * N_TRUE + e_p
    ukeys = np.unique(keys)
    uh = (ukeys // N_TRUE).astype(np.int64)
    up = (ukeys % N_TRUE).astype(np.int64)
    order = np.argsort(up, kind="stable")
    uh, up = uh[order], up[order]
    counts = np.bincount(up, minlength=N_TRUE)
    starts = np.zeros(N_TRUE + 1, np.int64)
    np.cumsum(counts, out=starts[1:])
    rank = np.arange(len(up)) - starts[up]

    # full center grid (replicated on every core); seg -> (t = seg//P, p)
    LF = max(4, int(counts.max()))
    ghf = np.full((N_TRUE, LF), -1, np.int64)
    ghf[up, rank] = uh
    valid = ghf >= 0
    gidx = np.clip(ghf, 0, None)
    # gf/gh: [P, NT4, LF] f32  (f pad -1, hid+1 pad 0)
    fg = np.where(valid, f[gidx], -1.0).astype(np.float32)
    hg = np.where(valid, (ghf + 1).astype(np.float32), 0.0).astype(np.float32)
    gfp = fg.reshape(NT4, P, LF).transpose(1, 0, 2)
    ghp = hg.reshape(NT4, P, LF).transpose(1, 0, 2)
    # gxt: [LF, D, NT4, P] bf16 (pad 0), edge-slot l on partitions
    gxv = np.where(valid[:, :, None], x[gidx], 0.0)        # [512, LF, D]
    gxt = gxv.reshape(NT4, P, LF, D).transpose(2, 3, 0, 1).astype(
        ml_dtypes.bfloat16)

    # per-core dealt member grids, d innermost: [P, NT4, LC, D+1]
    core = (rank % NC).astype(np.int64)
    slot = (rank // NC).astype(np.int64)
    LC = max(4, int(np.ceil(counts.max() / NC)))
    ghc = np.full((NC, N_TRUE, LC), -1, np.int64)
    ghc[core, up, slot] = uh

    aug = np.concatenate([x, f[:, None]], axis=1)       # [n_hit, 9]
    pad_row = np.zeros(D + 1, np.float32)
    pad_row[D] = -1.0

    in_maps = []
    for c in range(NC):
        g = ghc[c]
        staged = aug[np.clip(g, 0, None)]               # [512, LC, 9]
        staged[g < 0] = pad_row
        gg = np.ascontiguousarray(
            staged.reshape(NT4, P, LC, D + 1).transpose(1, 0, 2, 3))

        sl = slice(c * HPC, (c + 1) * HPC)
        x_loc = np.zeros((HPAD, D), np.float32)
        x_loc[:HPC] = x[sl]
        f_loc = np.zeros(HPAD, np.float32)
        f_loc[:HPC] = f[sl]
        y_loc = np.zeros(HPAD, np.float32)
        y_loc[:HPC] = y[sl].astype(np.float32)
        m_loc = np.zeros(HPAD, np.float32)
        m_loc[:HPC] = -1.0

        xt9 = np.empty((D + 1, HPAD), np.float32)
        xt9[:D] = x_loc.reshape(P, G49, D).transpose(2, 1, 0).reshape(D, HPAD)
        xt9[D] = 1.0
        in_maps.append({
            "gf": np.ascontiguousarray(gfp),
            "gh": np.ascontiguousarray(ghp),
            "gxt": np.ascontiguousarray(gxt),
            "xt9": np.ascontiguousarray(xt9),
            "xpm": np.ascontiguousarray(x_loc.reshape(P, G49 * D)),
            "fpm": np.ascontiguousarray(f_loc.reshape(P, G49)),
            "ypm": np.ascontiguousarray(y_loc.reshape(P, G49)),
            "nmask": np.ascontiguousarray(m_loc.reshape(P, G49)),
            "gg": gg,
        })
    return in_maps, LF, LC


def kernel(x, f, y, e_h, e_p, trace=False):
    in_maps, LF, LC = _prep(x, f, y, e_h, e_p)
    nc = _get_kernel(LF, LC)
    try:
        res = run_bass_kernel_spmd(nc, in_maps, core_ids=list(range(NC)),
                                   trace=trace)
    except ModuleNotFoundError:
        res = run_bass_kernel_spmd(nc, in_maps, core_ids=list(range(NC)),
                                   trace=False)
    parts = np.stack([res.results[c]["part"].sum(axis=0) for c in range(NC)])
    vacc = np.stack([res.results[c]["vacc"].sum() for c in range(NC)])
    vtot = vacc.sum() + parts[:, 4].sum()
    b2num = parts[:, 1].sum()
    nbkg = parts[:, 2].sum()
    fcsum = parts[0, 3]
    v = vtot / N_HIT
    b1 = 1.0 - fcsum / N_TRUE
    b2 = S_B * b2num / nbkg
    out = np.array([b1 + b2, v], dtype=np.float32)
    if trace:
        return out, res
    return out


# revision 62
# speedup vs baseline: 1.0087x; 1.0087x over previous
"""Object-condensation loss on 8 Trainium2 NeuronCores (Bass/Tile SPMD).

Hits are sharded 6250/core. The segment-max/argmax center search runs
replicated on every core over the unique-edge grid (f and hit-id rows
in f32 for an exact argmax; x rows in bf16 with edge-slots on
partitions). The winning-slot one-hot mask is PE-transposed so the
center-coordinate selection contracts on the tensor engine via
ones-vector matmuls. The dense [hits x 512] hinge term runs on PE
(f32r matmuls, one PSUM bank per 128-hit tile) with fused one-pass
consumers: ACT relu+scale+accumulate on single banks, DVE
(ps min 0)*(-q) scalar_tensor_tensor with accumulate over 3-bank
groups (GPSIMD cannot touch PSUM, so it handles the SBUF-side work:
mask TTs, member-correction products, background terms). Per-core
accumulators and partial scalars are DMA'd out raw and combined on
the host (the unshard step).

Host-side work is strictly index-driven data LAYOUT (slicing, padding,
reshapes, staging rows of [x|f] into edge grids by integer edge
indices, dtype casts) -- no floating-point arithmetic on the host.
"""
import os
import sys

sys.path.insert(0, "/opt/trn_rl_repo")

import numpy as np
import ml_dtypes

import concourse.bass as bass
import concourse.bacc as bacc
import concourse.tile as tile
from concourse import mybir
from concourse.bass_utils import run_bass_kernel_spmd
import jax as _jax
try:
    os.makedirs(os.path.expanduser("~/.cache/bass_jax_cache"), exist_ok=True)
    _jax.config.update("jax_compilation_cache_dir",
                       os.path.expanduser("~/.cache/bass_jax_cache"))
    _jax.config.update("jax_persistent_cache_min_entry_size_bytes", 0)
    _jax.config.update("jax_persistent_cache_min_compile_time_secs", 0)
except Exception:
    pass
from concourse.masks import make_identity

P = 128
NC = 8
N_HIT = 50000
N_TRUE = 512
D = 8
S_B = 1.0
Q_MIN = 0.5
HPC = N_HIT // NC          # 6250 hits per core
G49 = 49                   # free tiles per partition: 128*49 = 6272
HPAD = P * G49             # padded hits per core
NT4 = N_TRUE // P          # 4 segment tiles
F32 = mybir.dt.float32
F32R = mybir.dt.float32r
BF16 = mybir.dt.bfloat16
AX = mybir.AxisListType
OP = mybir.AluOpType
ACTF = mybir.ActivationFunctionType

N_WARM = 14                # PE p-state warmup matmuls

def build_kernel(LF, LC):
    """LF: slots/segment in the replicated center grid.
    LC: slots/segment in the per-core member-correction grid."""
    nc = bacc.Bacc("TRN2", target_bir_lowering=False, debug=False,
                   num_devices=NC)
    vec = nc.vector
    act = nc.scalar
    gps = nc.gpsimd

    # ---------------- I/O ----------------
    gf_in = nc.dram_tensor("gf", [P, NT4, LF], F32, kind="ExternalInput")
    gh_in = nc.dram_tensor("gh", [P, NT4, LF], F32, kind="ExternalInput")
    gxt_in = nc.dram_tensor("gxt", [LF, D, NT4, P], BF16, kind="ExternalInput")
    xt9_in = nc.dram_tensor("xt9", [D + 1, HPAD], F32R, kind="ExternalInput")
    xpm_in = nc.dram_tensor("xpm", [P, G49 * D], F32, kind="ExternalInput")
    fpm_in = nc.dram_tensor("fpm", [P, G49], F32, kind="ExternalInput")
    ypm_in = nc.dram_tensor("ypm", [P, G49], F32, kind="ExternalInput")
    nmask_in = nc.dram_tensor("nmask", [P, G49], F32, kind="ExternalInput")
    gg_in = nc.dram_tensor("gg", [P, NT4, LC, D + 1], F32, kind="ExternalInput")
    part_out = nc.dram_tensor("part", [P, NC], F32, kind="ExternalOutput")
    vacc_out = nc.dram_tensor("vacc", [P, 3, G49], F32, kind="ExternalOutput")

    with tile.TileContext(nc) as tc:
        with (
            tc.tile_pool(name="sbuf", bufs=1) as sb,
            tc.tile_pool(name="sjunk", bufs=2) as sj,
            tc.tile_pool(name="psum", bufs=2, space="PSUM") as pp,
            tc.tile_pool(name="psa", bufs=2, space="PSUM") as pa,
        ):
            # ------------- loads (gpsimd queue: cheap issue) -------------
            gf = sb.tile([P, NT4, LF], F32)
            nc.sync.dma_start(out=gf[:], in_=gf_in[:])
            gh = sb.tile([P, NT4, LF], F32)
            nc.sync.dma_start(out=gh[:], in_=gh_in[:])
            ident = sb.tile([P, P], F32)
            make_identity(nc, ident[:])
            cm1 = sb.tile([P, 1], F32)
            gps.memset(cm1[:], -1.0)
            actwarm = sb.tile([P, 1], F32)
            act.activation(out=actwarm[:], in_=cm1[:], func=ACTF.Ln,
                           scale=-1.0)
            gxt = sb.tile([LF, D, NT4, P], BF16)
            nc.sync.dma_start(out=gxt[:], in_=gxt_in[:])
            onesb = sb.tile([LF, 1], BF16)
            gps.memset(onesb[:], 1.0)
            xt = sb.tile([D + 2, HPAD], F32R)
            nc.sync.dma_start(out=xt[0:D + 1, :], in_=xt9_in[:])
            xpm = sb.tile([P, G49, D], F32)
            nc.sync.dma_start(out=xpm[:],
                              in_=xpm_in[:].rearrange("p (g d) -> p g d", d=D))
            fpm = sb.tile([P, G49], F32)
            nc.sync.dma_start(out=fpm[:], in_=fpm_in[:])
            ypm = sb.tile([P, G49], F32)
            nc.sync.dma_start(out=ypm[:], in_=ypm_in[:])
            nmask = sb.tile([P, G49], F32)
            nc.sync.dma_start(out=nmask[:], in_=nmask_in[:])
            gg = sb.tile([P, NT4, LC, D + 1], F32)
            nc.sync.dma_start(out=gg[:], in_=gg_in[:])

            # ------------- PE p-state warmup (small f32 matmuls) -------------
            for w in range(N_WARM):
                wp = pa.tile([P, 512], F32, space="PSUM", tag="da")
                nc.tensor.matmul(out=wp[:, 0:32], lhsT=ident[:],
                                 rhs=ident[:, 0:32], start=True, stop=True)

            # ------------- center search: segment max + argmax -------------
            cfe = gf[:]                                 # [P, 4, LF] f
            chp = gh[:]                                 # [P, 4, LF] hit+1
            fmax = sb.tile([P, NT4], F32)
            vec.tensor_reduce(out=fmax[:], in_=cfe, axis=AX.X, op=OP.max)
            xsq = sj.tile([P, G49, D], F32, tag="xsq")
            act.activation(out=xsq[:], in_=xpm[:], func=ACTF.Square)
            x2s1 = sj.tile([P, G49, 4], F32, tag="x2s1")
            gps.tensor_tensor(out=x2s1[:], in0=xsq[:, :, 0:4],
                              in1=xsq[:, :, 4:8], op=OP.add)
            x2s2 = sj.tile([P, G49, 2], F32, tag="x2s2")
            gps.tensor_tensor(out=x2s2[:], in0=x2s1[:, :, 0:2],
                              in1=x2s1[:, :, 2:4], op=OP.add)
            x2pmt = sb.tile([P, G49, 1], F32)
            gps.tensor_tensor(out=x2pmt[:], in0=x2s2[:, :, 0:1],
                              in1=x2s2[:, :, 1:2], op=OP.add)
            eq1 = sj.tile([P, NT4, LF], F32, tag="eq1")
            vec.tensor_tensor(out=eq1[:], in0=cfe,
                              in1=fmax[:].to_broadcast([P, NT4, LF]),
                              op=OP.is_equal)
            cnd = sj.tile([P, NT4, LF], F32, tag="cnd")
            vec.tensor_tensor(out=cnd[:], in0=eq1[:], in1=chp, op=OP.mult)
            candp = sb.tile([P, NT4], F32)
            vec.tensor_reduce(out=candp[:], in_=cnd[:], axis=AX.X, op=OP.max)
            m3 = sb.tile([P, NT4, LF], F32)
            vec.tensor_tensor(out=m3[:], in0=chp,
                              in1=candp[:].to_broadcast([P, NT4, LF]),
                              op=OP.is_equal)

            # qc chain (ACT-heavy, off the DVE critical path)
            qcb = sj.tile([P, NT4], F32, tag="qcb")
            qcr = sj.tile([P, NT4], F32, tag="qcr")
            qcl = sj.tile([P, NT4], F32, tag="qcl")
            qcs = sj.tile([P, NT4], F32, tag="qcs")
            vec.tensor_scalar(qcb[:], fmax[:], -1.0, 1.0, OP.mult, OP.add)
            vec.reciprocal(qcr[:], qcb[:])
            act.activation(out=qcl[:], in_=qcr[:], func=ACTF.Ln,
                           scale=2.0, bias=cm1[:])
            act.activation(out=qcs[:], in_=qcl[:], func=ACTF.Square, scale=0.5)
            qc = sb.tile([P, NT4], F32)
            vec.tensor_scalar(qc[:], qcs[:], Q_MIN, None, OP.add)
            n2qc = sb.tile([P, NT4], F32)
            vec.tensor_scalar(n2qc[:], qc[:], -2.0, None, OP.mult)
            fcsum = sb.tile([P, 1], F32)
            vec.tensor_reduce(out=fcsum[:], in_=fmax[:], axis=AX.X, op=OP.add)

            # d-selection: transpose m3 to [LF, NT4*P]; mask the
            # slot-major x grid (bf16 2x TTs); contract slots on PE via
            # ones-vector matmuls -> xc[seg, t*8+d] lands on partitions.
            m3w3 = pp.tile([P, 3, 512], F32, space="PSUM", tag="dense")
            m3ps = m3w3[0:LF, 0, :]
            for t in range(NT4):
                nc.tensor.transpose(out=m3ps[:, t * P:(t + 1) * P],
                                    in_=m3[:, t, :], identity=ident[:])
            m3t = sb.tile([LF, NT4, P], BF16)
            vec.tensor_copy(out=m3t[:], in_=m3ps.rearrange(
                "l (t p) -> l t p", p=P))
            m3v = m3t[:]
            xcw = pa.tile([P, 512], F32, space="PSUM", tag="da")
            x2sb = sb.tile([G49, P], F32R)
            for d in range(D):
                md = sj.tile([LF, NT4, P], BF16, tag=f"md{d % 4}")
                eng = gps if d < 2 else vec
                eng.tensor_tensor(out=md[:], in0=gxt[:, d, :, :], in1=m3v,
                                  op=OP.mult)
                for t in range(NT4):
                    nc.tensor.matmul(out=xcw[:, t * D + d:t * D + d + 1],
                                     lhsT=md[:, t, :], rhs=onesb[:],
                                     start=True, stop=True)
                if d == 2:
                    x2ps = pa.tile([P, 512], F32, space="PSUM", tag="da")
                    nc.tensor.transpose(out=x2ps[0:G49, 0:P],
                                        in_=x2pmt[:, :, 0],
                                        identity=ident[:])
                if d == 3:
                    act.activation(out=x2sb[:], in_=x2ps[0:G49, 0:P],
                                   func=ACTF.Copy)
                    nc.sync.dma_start(out=xt[D + 1:D + 2, :], in_=x2sb[:])
            xcf = sb.tile([P, NT4 * D], F32)
            vec.tensor_copy(out=xcf[:], in_=xcw[:, 0:NT4 * D])
            xc = xcf[:].rearrange("p (t d) -> p t d", d=D)

            # ------------- background terms (pool) -------------
            bkg = sj.tile([P, G49], F32, tag="bkg")
            vec.tensor_scalar(bkg[:], ypm[:], -1.0, None, OP.is_equal)
            bf = sj.tile([P, G49], F32, tag="bf")
            b2num = sb.tile([P, 1], F32)
            gps.tensor_tensor(out=bf[:], in0=bkg[:], in1=fpm[:], op=OP.mult)
            vec.tensor_reduce(out=b2num[:], in_=bf[:], axis=AX.X, op=OP.add)
            nbkg = sb.tile([P, 1], F32)
            vec.tensor_reduce(out=nbkg[:], in_=bkg[:], axis=AX.X, op=OP.add)

            # ------------- ctil = [-2qc*xc | qc*(c2-1) | qc] -> cT ------
            csq = sj.tile([P, NT4 * D], F32, tag="csq")
            vec.tensor_tensor(out=csq[:], in0=xcf[:], in1=xcf[:], op=OP.mult)
            c2 = sb.tile([P, NT4], F32)
            vec.tensor_reduce(out=c2[:], in_=csq[:].rearrange(
                "p (t d) -> p t d", d=D), axis=AX.X, op=OP.add)
            ctil = sb.tile([P, NT4, D + 2], F32)
            vec.tensor_tensor(
                out=ctil[:, :, 0:D], in0=xc,
                in1=n2qc[:].to_broadcast([P, NT4, D]), op=OP.mult)
            vec.scalar_tensor_tensor(
                out=ctil[:, :, D:D + 1].rearrange("p t o -> p (t o)"),
                in0=c2[:], scalar=-1.0, in1=qc[:], op0=OP.add, op1=OP.mult)
            vec.tensor_copy(out=ctil[:, :, D + 1:D + 2].rearrange("p t o -> p (t o)"),
                            in_=qc[:])
            ctps = pa.tile([P, 512], F32, space="PSUM", tag="da")
            for t in range(NT4):
                nc.tensor.transpose(out=ctps[0:D + 2, t * P:(t + 1) * P],
                                    in_=ctil[:, t, :], identity=ident[:])
            cT = sb.tile([D + 2, N_TRUE], F32R)
            act.activation(out=cT[:], in_=ctps[0:D + 2, :], func=ACTF.Copy)

            # ------------- per-hit q (negated, masked) -------------
            qb = sj.tile([P, G49], F32, tag="qb")
            qr = sj.tile([P, G49], F32, tag="qr")
            ql = sj.tile([P, G49], F32, tag="ql")
            qs = sj.tile([P, G49], F32, tag="qs")
            vec.tensor_scalar(qb[:], fpm[:], -1.0, 1.0, OP.mult, OP.add)
            vec.reciprocal(qr[:], qb[:])
            act.activation(out=ql[:], in_=qr[:], func=ACTF.Ln,
                           scale=2.0, bias=cm1[:])
            act.activation(out=qs[:], in_=ql[:], func=ACTF.Square, scale=0.5)
            negq = sb.tile([P, G49], F32)
            vec.scalar_tensor_tensor(out=negq[:], in0=qs[:], scalar=Q_MIN,
                                     in1=nmask[:], op0=OP.add, op1=OP.mult)


            # ------------- dense phase: one fused consumer per tile ------
            acc_a = sb.tile([P, G49], F32)
            acc_d = sb.tile([P, 16], F32)
            gps.memset(acc_a[:], 0.0)
            gps.memset(acc_d[:], 0.0)

            # work items: 9 DVE 3-bank groups (27 tiles) + 22 ACT singles
            items = []
            gD, gA = 0, 27
            acounts = [2, 2, 2, 2, 2, 2, 2, 2, 2]
            for _ in range(4):
                items.append(("A", gA, 0))
                gA += 1
            for i in range(9):
                items.append(("D", gD, i))
                gD += 3
                for _ in range(acounts[i]):
                    items.append(("A", gA, 0))
                    gA += 1

            def dense_item(kind, g0, slot):
                if kind == "D":
                    ps3 = pp.tile([P, 3, 512], F32, space="PSUM", tag="dense")
                    for j in range(3):
                        g = g0 + j
                        nc.tensor.matmul(out=ps3[:, j, :],
                                         lhsT=xt[:, g * P:(g + 1) * P],
                                         rhs=cT[:], start=True, stop=True)
                    nqv = negq[:, g0:g0 + 3].rearrange(
                        "p j -> p j ()").to_broadcast([P, 3, N_TRUE])
                    jd = sj.tile([P, 3, N_TRUE], F32, tag="jd")
                    vec.scalar_tensor_tensor(
                        out=jd[:], in0=ps3[:], scalar=0.0, in1=nqv,
                        op0=OP.min, op1=OP.mult,
                        accum_out=acc_d[:, slot:slot + 1])
                else:
                    ps = pa.tile([P, 512], F32, space="PSUM", tag="da")
                    nc.tensor.matmul(out=ps[:], lhsT=xt[:, g0 * P:(g0 + 1) * P],
                                     rhs=cT[:], start=True, stop=True)
                    act.activation(out=ps[:], in_=ps[:], func=ACTF.Relu,
                                   scale=negq[:, g0:g0 + 1],
                                   accum_out=acc_a[:, g0:g0 + 1])

            for kind, g0, slot in items[:8]:
                dense_item(kind, g0, slot)

            # ------------- member correction (mid-dense; d-inner grid) ------
            dif = sj.tile([P, NT4, LC, D], F32, tag="dif")
            vec.tensor_tensor(
                out=dif[:], in0=gg[:, :, :, 0:D],
                in1=xcf[:].rearrange("p (t d) -> p t () d", d=D).to_broadcast(
                    [P, NT4, LC, D]),
                op=OP.subtract)
            dsq = sj.tile([P, NT4, LC, D], F32, tag="dsq")
            vec.tensor_tensor(out=dsq[:], in0=dif[:], in1=dif[:], op=OP.mult)
            dste = sj.tile([P, NT4, LC], F32, tag="dste")
            vec.tensor_reduce(out=dste[:], in_=dsq[:], axis=AX.X, op=OP.add)
            # hinge correction weight: dist + min(dist-1, 0)
            we0 = sj.tile([P, NT4, LC], F32, tag="we0")
            vec.tensor_scalar(we0[:], dste[:], 1.0, 0.0, OP.subtract, OP.min)
            we = sj.tile([P, NT4, LC], F32, tag="we")
            gps.tensor_tensor(out=we[:], in0=we0[:], in1=dste[:], op=OP.add)
            wq = sj.tile([P, NT4, LC], F32, tag="wq")
            gps.tensor_tensor(out=wq[:], in0=we[:],
                              in1=qc[:].to_broadcast([P, NT4, LC]), op=OP.mult)
            # qe = (atanh(fe)^2 + Q_MIN) * (fe >= 0)
            fe = gg[:, :, :, D]                          # [P, 4, LC]
            eb = sj.tile([P, NT4, LC], F32, tag="eb")
            ec = sj.tile([P, NT4, LC], F32, tag="ec")
            er = sj.tile([P, NT4, LC], F32, tag="er")
            el = sj.tile([P, NT4, LC], F32, tag="el")
            es = sj.tile([P, NT4, LC], F32, tag="es")
            ev = sj.tile([P, NT4, LC], F32, tag="ev")
            # clamp pad slots (fe = -1) to f = 0 so Ln stays finite
            vec.tensor_scalar(ec[:], fe, 0.0, -1.0, OP.max, OP.mult)
            vec.tensor_scalar(eb[:], ec[:], 1.0, None, OP.add)
            vec.reciprocal(er[:], eb[:])
            act.activation(out=el[:], in_=er[:], func=ACTF.Ln,
                           scale=2.0, bias=cm1[:])
            act.activation(out=es[:], in_=el[:], func=ACTF.Square, scale=0.5)
            vec.tensor_scalar(ev[:], fe, 0.0, None, OP.is_ge)
            qe = sj.tile([P, NT4, LC], F32, tag="qe")
            vec.scalar_tensor_tensor(out=qe[:], in0=es[:], scalar=Q_MIN,
                                     in1=ev[:], op0=OP.add, op1=OP.mult)
            wfin = sj.tile([P, NT4, LC], F32, tag="wfin")
            corr = sb.tile([P, 1], F32)
            gps.tensor_tensor(out=wfin[:], in0=wq[:], in1=qe[:], op=OP.mult,
                              )
            vec.tensor_reduce(out=corr[:], in_=wfin[:], axis=AX.XY, op=OP.add)

            for kind, g0, slot in items[8:]:
                dense_item(kind, g0, slot)

            # ------------- partial outputs [P, 8] -------------
            stk = sb.tile([P, NC], F32)
            gps.memset(stk[:], 0.0)
            gps.tensor_copy(out=stk[:, 1:2], in_=b2num[:])
            gps.tensor_copy(out=stk[:, 2:3], in_=nbkg[:])
            gps.tensor_copy(out=stk[:, 3:4], in_=fcsum[:])
            gps.tensor_copy(out=stk[:, 4:5], in_=corr[:])
            nc.sync.dma_start(out=part_out[:], in_=stk[:])
            act.dma_start(out=vacc_out[:, 0, 0:G49], in_=acc_a[:])
            nc.sync.dma_start(out=vacc_out[:, 1, 0:16], in_=acc_d[:])

    nc.compile()
    return nc


_CACHE = {}


def _get_kernel(LF, LC):
    key = (LF, LC)
    if key not in _CACHE:
        _CACHE[key] = build_kernel(LF, LC)
    return _CACHE[key]


def _prep(x, f, y, e_h, e_p):
    x = np.asarray(x, np.float32)
    f = np.asarray(f, np.float32)
    y = np.asarray(y).astype(np.int64)
    e_h = np.asarray(e_h).astype(np.int64)
    e_p = np.asarray(e_p).astype(np.int64)

    keys = e_h * N_TRUE + e_p
    ukeys = np.unique(keys)
    uh = (ukeys // N_TRUE).astype(np.int64)
    up = (ukeys % N_TRUE).astype(np.int64)
    order = np.argsort(up, kind="stable")
    uh, up = uh[order], up[order]
    counts = np.bincount(up, minlength=N_TRUE)
    starts = np.zeros(N_TRUE + 1, np.int64)
    np.cumsum(counts, out=starts[1:])
    rank = np.arange(len(up)) - starts[up]

    # full center grid (replicated on every core); seg -> (t = seg//P, p)
    LF = max(4, int(counts.max()))
    ghf = np.full((N_TRUE, LF), -1, np.int64)
    ghf[up, rank] = uh
    valid = ghf >= 0
    gidx = np.clip(ghf, 0, None)
    # gf/gh: [P, NT4, LF] f32  (f pad -1, hid+1 pad 0)
    fg = np.where(valid, f[gidx], -1.0).astype(np.float32)
    hg = np.where(valid, (ghf + 1).astype(np.float32), 0.0).astype(np.float32)
    gfp = fg.reshape(NT4, P, LF).transpose(1, 0, 2)
    ghp = hg.reshape(NT4, P, LF).transpose(1, 0, 2)
    # gxt: [LF, D, NT4, P] bf16 (pad 0), edge-slot l on partitions
    gxv = np.where(valid[:, :, None], x[gidx], 0.0)        # [512, LF, D]
    gxt = gxv.reshape(NT4, P, LF, D).transpose(2, 3, 0, 1).astype(
        ml_dtypes.bfloat16)

    # per-core dealt member grids, d innermost: [P, NT4, LC, D+1]
    core = (rank % NC).astype(np.int64)
    slot = (rank // NC).astype(np.int64)
    LC = max(4, int(np.ceil(counts.max() / NC)))
    ghc = np.full((NC, N_TRUE, LC), -1, np.int64)
    ghc[core, up, slot] = uh

    aug = np.concatenate([x, f[:, None]], axis=1)       # [n_hit, 9]
    pad_row = np.zeros(D + 1, np.float32)
    pad_row[D] = -1.0

    in_maps = []
    for c in range(NC):
        g = ghc[c]
        staged = aug[np.clip(g, 0, None)]               # [512, LC, 9]
        staged[g < 0] = pad_row
        gg = np.ascontiguousarray(
            staged.reshape(NT4, P, LC, D + 1).transpose(1, 0, 2, 3))

        sl = slice(c * HPC, (c + 1) * HPC)
        x_loc = np.zeros((HPAD, D), np.float32)
        x_loc[:HPC] = x[sl]
        f_loc = np.zeros(HPAD, np.float32)
        f_loc[:HPC] = f[sl]
        y_loc = np.zeros(HPAD, np.float32)
        y_loc[:HPC] = y[sl].astype(np.float32)
        m_loc = np.zeros(HPAD, np.float32)
        m_loc[:HPC] = -1.0

        xt9 = np.empty((D + 1, HPAD), np.float32)
        xt9[:D] = x_loc.reshape(P, G49, D).transpose(2, 1, 0).reshape(D, HPAD)
        xt9[D] = 1.0
        in_maps.append({
            "gf": np.ascontiguousarray(gfp),
            "gh": np.ascontiguousarray(ghp),
            "gxt": np.ascontiguousarray(gxt),
            "xt9": np.ascontiguousarray(xt9),
            "xpm": np.ascontiguousarray(x_loc.reshape(P, G49 * D)),
            "fpm": np.ascontiguousarray(f_loc.reshape(P, G49)),
            "ypm": np.ascontiguousarray(y_loc.reshape(P, G49)),
            "nmask": np.ascontiguousarray(m_loc.reshape(P, G49)),
            "gg": gg,
        })
    return in_maps, LF, LC


def kernel(x, f, y, e_h, e_p, trace=False):
    in_maps, LF, LC = _prep(x, f, y, e_h, e_p)
    nc = _get_kernel(LF, LC)
    try:
        res = run_bass_kernel_spmd(nc, in_maps, core_ids=list(range(NC)),
                                   trace=trace)
    except ModuleNotFoundError:
        res = run_bass_kernel_spmd(nc, in_maps, core_ids=list(range(NC)),
                                   trace=False)
    parts = np.stack([res.results[c]["part"].sum(axis=0) for c in range(NC)])
    vacc = np.stack([res.results[c]["vacc"].sum() for c in range(NC)])
    vtot = vacc.sum() + parts[:, 4].sum()
    b2num = parts[:, 1].sum()
    nbkg = parts[:, 2].sum()
    fcsum = parts[0, 3]
    v = vtot / N_HIT
    b1 = 1.0 - fcsum / N_TRUE
    b2 = S_B * b2num / nbkg
    out = np.array([b1 + b2, v], dtype=np.float32)
    if trace:
        return out, res
    return out
